# revision 28
# baseline (speedup 1.0000x reference)
"""Trainium2 Bass kernel for nn_BasicBlock_88665304858673 (spiking BasicBlock).

Computation (dead code removed -- mem2/o2/m2, memd/od and inp_u never reach
the outputs):

  per time step t (T=4):
    I1_t   = conv1(x_t)            3x3 stride2 pad1, 256->512, BN-folded
    mem1  += I1_t ; o1_t = (mem1 >= vth1) ; mem1 -= o1_t*vth1 ; mask1 |= o1_t
    out_s_t = conv2(o1_t) + convd(x_t)     (3x3 s1 p1 and 1x1 s2)
    memf  += out_s_t ; o3_t = (memf >= vth_if) ; memf -= o3_t*vth_if ; mask3 |= o3_t
  outputs: o3_3, out_s_3, and the ANN branch
    a     = relu(conv1(inp_c)) * mask1
    out_c = relu(conv2(a) + convd(inp_c)) * mask3

Sharding: data-parallel over batch B=32 -> 8 cores x 4 images (2 pairs of 2;
matmul moving dim N = 2*196 = 392).

Matmul dtype plan (1 cycle/col dtypes only; fp32 is 4 cyc/col):
  conv1  : bf16 3-term split  xh*wh + xh*wl + xl*wh   (~2^-17 accurate; conv1
           feeds the mem1 threshold whose spike flips amplify through conv2,
           so coarser terms are NOT enough -- verified in simulation: fp8
           corrections and 2-term variants all blow past the rel-err gate)
  conv2  : fp16 weights (11-bit mantissa) x bf16 spike moving operand. o1 is
           binary {0,1} => exact in bf16; only w2 sees the 2^-11 rounding
           (sim: total rel err 1.89e-2 vs numpy oracle, 2e-2 gate). fp16
           stationary x bf16 moving is legal on TRN2 (probed; only 32-bit x
           non-32-bit mixing is rejected by the BIR verifier)
  convd  : f32r x f32r single term (2^-12, small magnitude)
  ANN    : conv1 bf16 single (reuses resident w1 hi), conv2 fp16 x fp16
           ('a' activations stored fp16), convd f32r; no thresholds
           downstream.

All conv1/ANN-conv1 moving operands are per-tap FULLY CONTIGUOUS copies:
the host pre-extracts, for each of the 9 taps, the 196 stride-2 window
pixels per image from the zero-padded 30x30 frame (layout [tap][img][196],
392-element single-run matmuls). Strided plane views cost ~2 PE cycles per
run transition (~15 ns per 392-col matmul measured), and the contiguous
layout removes them. fp16 w2 (37KB vs 74KB f32r) + bf16 o1 rings (21KB vs
41KB) pay for the larger x tiles.

o1 spikes are stored as 3 kx-pre-shifted zero-padded bf16 copies so every
conv2 tap reads two fully contiguous 196-element runs (one per image).
ANN 'a' activations get their own fp16 rings of the same layout.

Weights stay resident in SBUF for the whole kernel (loaded once, cok-major
so per-cok DMA chunks unlock the first conv groups early): w1 hi/lo bf16,
w2 fp16, wd f32r. p-state warmup matmuls keep the PE clock up through the
startup DMA window, which is HBM-bandwidth-bound: the first conv group's
operands (xh + w1h cok0) stream ahead of everything else, and warmup is
sized to end at their arrival (~14.5us). The last pair's ANN groups run
before its spike-t3 conv2 group so only the short scanF(3) + mask-gate +
store tail follows the final matmul (output stores split across the Sync
and Scalar DMA issuers). NOTE: GpSimd/Pool elementwise ops measure ~29x
slower than Vector ([128,392] IS_GE: 7.7us vs 267ns) -- never offload
tail ops there.

Measured: 1266927ns (fp32 naive) -> 621567ns (bf16/f32r strided planes) ->
~587400ns (this version; tensor stream gap-free at ~166ns per 392-col
matmul vs 163.3ns theoretical). Rel err 1.87e-2 vs a local numpy oracle
whose ref-vs-ref spike flips inflate it; ~1.3e-2 expected vs the jax
oracle (gate 2e-2).
"""

import numpy as np
import ml_dtypes

EPS = 1e-5
NCORES = 8
BPC = 4          # images per core
NPAIR = 2        # image pairs per core
NIMG = 2         # images per pair
PIX = 196
NN = NIMG * PIX  # moving dim: 392
NTAP = 9
XF = NTAP * NN   # per-cik tapped x layout: [tap][img][196] = 3528

_CACHE = {}
TRACE = False
LAST_RESULT = None

# o1/a ring layout: 3 kx-shifted copies, each [2 img, 15 rows, 14 cols]
# zero-padded, plus a 28-elem zero tail: copy kx at [kx*420 + b*210 + y*14
# + x] holds v[b, y-1, x-1+kx].  A tap (ky,kx) then reads TWO CONTIGUOUS
# 196-elem runs (one per image).  Vertical overruns (row 15) land on the
# next block's zero pad row.  Double-buffered by timestep parity.
RING = 1288


def _build(cfg):
    """cfg = (bias1_any, bias2_any, vth1_scalar_or_None, vthf_scalar_or_None)"""
    import concourse.bacc as bacc
    import concourse.mybir as mybir
    import concourse.tile as tile

    F32 = mybir.dt.float32
    F32R = mybir.dt.float32r
    BF16 = mybir.dt.bfloat16
    FP16 = mybir.dt.float16
    Alu = mybir.AluOpType
    Act = mybir.ActivationFunctionType
    bias1_any, bias2_any, vth1_c, vthf_c = cfg

    nc = bacc.Bacc(None, target_bir_lowering=False)

    W1Hd = nc.dram_tensor("W1H", [128, 2 * 9 * 512], BF16, kind="ExternalInput")
    W1Ld = nc.dram_tensor("W1L", [128, 2 * 9 * 512], BF16, kind="ExternalInput")
    W2Hd = nc.dram_tensor("W2H", [128, 4 * 9 * 512], FP16, kind="ExternalInput")
    WDRd = nc.dram_tensor("WDR", [128, 2 * 512], F32R, kind="ExternalInput")
    XTHd = nc.dram_tensor("XTH", [NPAIR, 4, 2, 128, XF], BF16,
                          kind="ExternalInput")
    XTLd = nc.dram_tensor("XTL", [NPAIR, 4, 2, 128, XF], BF16,
                          kind="ExternalInput")
    XDd = nc.dram_tensor("XD", [NPAIR, 4, 2, 128, NN], F32R,
                         kind="ExternalInput")
    XCd = nc.dram_tensor("XC", [NPAIR, 2, 128, XF], BF16,
                         kind="ExternalInput")
    XCDd = nc.dram_tensor("XCD", [NPAIR, 2, 128, NN], F32R,
                          kind="ExternalInput")
    if bias1_any:
        B1Hd = nc.dram_tensor("B1H", [1, 512], BF16, kind="ExternalInput")
        B1Ld = nc.dram_tensor("B1L", [1, 512], BF16, kind="ExternalInput")
    if bias2_any:
        B2Hd = nc.dram_tensor("B2H", [1, 512], F32R, kind="ExternalInput")
        B2Ld = nc.dram_tensor("B2L", [1, 512], F32R, kind="ExternalInput")
        ONERd = nc.dram_tensor("ONER", [1, NN], F32R, kind="ExternalInput")
    if vth1_c is None:
        V1d = nc.dram_tensor("VTH1R", [128, 4 * NN], F32, kind="ExternalInput")
    if vthf_c is None:
        VFd = nc.dram_tensor("VTHFR", [128, 4 * NN], F32, kind="ExternalInput")
    O3d = nc.dram_tensor("O3", [NPAIR, 128, 4 * NN], F32, kind="ExternalOutput")
    IUd = nc.dram_tensor("IU", [NPAIR, 4, 128, NN], F32, kind="ExternalOutput")
    OCd = nc.dram_tensor("OC", [NPAIR, 128, 4 * NN], F32, kind="ExternalOutput")

    with tile.TileContext(nc) as tc:
        with tc.tile_pool(name="wpool", bufs=1) as wp, \
             tc.tile_pool(name="xpool", bufs=2) as xp, \
             tc.tile_pool(name="spool", bufs=1) as st, \
             tc.tile_pool(name="ostr", bufs=2) as ostr, \
             tc.tile_pool(name="xdpool", bufs=2) as xdp, \
             tc.tile_pool(name="iupool", bufs=1) as iup, \
             tc.tile_pool(name="pspool", bufs=4, space="PSUM") as pp:

            w1h = wp.tile([128, 2 * 9 * 512], BF16, name="w1h")
            w1l = wp.tile([128, 2 * 9 * 512], BF16, name="w1l")
            w2h = wp.tile([128, 4 * 9 * 512], FP16, name="w2h")
            wdr = wp.tile([128, 2 * 512], F32R, name="wdr")

            mem1 = [st.tile([128, NN], F32, name=f"mem1_{k}") for k in range(4)]
            memf = [st.tile([128, NN], F32, name=f"memf_{k}") for k in range(4)]
            U8 = mybir.dt.uint8
            mask1 = [st.tile([128, NN], U8, name=f"mask1_{k}") for k in range(4)]
            mask3 = [st.tile([128, NN], U8, name=f"mask3_{k}") for k in range(4)]
            # o1 spike rings (bf16), double-buffered by timestep parity
            o1p = [[st.tile([128, RING], BF16, name=f"o1p_{k}_{par}")
                    for par in range(2)] for k in range(4)]
            # ANN 'a' activation rings (fp16)
            aap = [st.tile([128, RING], FP16, name=f"aap_{k}")
                   for k in range(4)]

            if bias1_any:
                ones_b = st.tile([1, NN], BF16, name="ones_b")
                nc.vector.memset(ones_b[:], 1.0)
                b1h = st.tile([1, 512], BF16, name="b1h")
                nc.sync.dma_start(out=b1h[:], in_=B1Hd[:])
                b1l = st.tile([1, 512], BF16, name="b1l")
                nc.sync.dma_start(out=b1l[:], in_=B1Ld[:])
            if bias2_any:
                ones_r = st.tile([1, NN], F32R, name="ones_r")
                nc.sync.dma_start(out=ones_r[:], in_=ONERd[:])
                b2h = st.tile([1, 512], F32R, name="b2h")
                nc.sync.dma_start(out=b2h[:], in_=B2Hd[:])
                b2l = st.tile([1, 512], F32R, name="b2l")
                nc.sync.dma_start(out=b2l[:], in_=B2Ld[:])
            if vth1_c is None:
                v1t = st.tile([128, 4 * NN], F32, name="v1t")
                nc.sync.dma_start(out=v1t[:], in_=V1d[:])
            if vthf_c is None:
                vft = st.tile([128, 4 * NN], F32, name="vft")
                nc.sync.dma_start(out=vft[:], in_=VFd[:])

            def load_x(pair, t):
                # x loads issue from the Scalar sequencer (2nd HWDGE issuer)
                # so they don't serialize behind weight/output DMAs on Sync.
                xh, xl, xd = [], [], []
                for cik in range(2):
                    th = xp.tile([128, XF], BF16,
                                 name=f"xh_{pair}_{t}_{cik}", tag=f"xh{cik}")
                    nc.sync.dma_start(out=th[:], in_=XTHd[pair, t, cik])
                    xh.append(th)
                for cik in range(2):
                    tl = xp.tile([128, XF], BF16,
                                 name=f"xl_{pair}_{t}_{cik}", tag=f"xl{cik}")
                    nc.sync.dma_start(out=tl[:], in_=XTLd[pair, t, cik])
                    xl.append(tl)
                for cik in range(2):
                    td = xdp.tile([128, NN], F32R,
                                  name=f"xd_{pair}_{t}_{cik}", tag=f"xd{cik}")
                    nc.sync.dma_start(out=td[:], in_=XDd[pair, t, cik])
                    xd.append(td)
                return xh, xl, xd

            def load_xc(pair):
                xc, xcd = [], []
                for cik in range(2):
                    t = xp.tile([128, XF], BF16,
                                name=f"xc_{pair}_{cik}", tag=f"xh{cik}")
                    nc.sync.dma_start(out=t[:], in_=XCd[pair, cik])
                    xc.append(t)
                    td = xdp.tile([128, NN], F32R,
                                  name=f"xcd_{pair}_{cik}", tag=f"xd{cik}")
                    nc.sync.dma_start(out=td[:], in_=XCDd[pair, cik])
                    xcd.append(td)
                return xc, xcd

            def conv1_group(xh, xl, cok):
                """18 taps x 3 bf16 terms accumulating dense into one psum.
                Term-major order so the first 18 matmuls need only xh+w1h --
                shrinks the startup DMA critical set. All moving operands
                are contiguous 392-element tap slices."""
                ps = pp.tile([128, NN], F32, name="ps1", tag="ps1")
                total = 54 + (2 if bias1_any else 0)
                n = 0
                for wsel, xsel in ((0, 0), (1, 0), (0, 1)):
                    xt = xh if xsel == 0 else xl
                    wt = w1h if wsel == 0 else w1l
                    for cik in range(2):
                        for ti in range(9):
                            ws = ((cok * 2 + cik) * 9 + ti) * 128
                            r_t = xt[cik][:, ti * NN:(ti + 1) * NN]
                            nc.tensor.matmul(ps[:], wt[:, ws:ws + 128], r_t,
                                             start=(n == 0),
                                             stop=(n == total - 1),
                                             skip_group_check=True)
                            n += 1
                if bias1_any:
                    for b in (b1h, b1l):
                        nc.tensor.matmul(ps[:], b[0:1, cok * 128:(cok + 1) * 128],
                                         ones_b[:], start=False,
                                         stop=(n == total - 1),
                                         skip_group_check=True)
                        n += 1
                return ps

            def conv1_group_ann(xc, cok):
                """ANN conv1: single-term bf16 on resident w1 hi."""
                ps = pp.tile([128, NN], F32, name="ps1", tag="ps1")
                total = 18 + (2 if bias1_any else 0)
                n = 0
                for cik in range(2):
                    for ti in range(9):
                        ws = ((cok * 2 + cik) * 9 + ti) * 128
                        rhs = xc[cik][:, ti * NN:(ti + 1) * NN]
                        nc.tensor.matmul(ps[:], w1h[:, ws:ws + 128], rhs,
                                         start=(n == 0),
                                         stop=(n == total - 1),
                                         skip_group_check=True)
                        n += 1
                if bias1_any:
                    for b in (b1h, b1l):
                        nc.tensor.matmul(ps[:], b[0:1, cok * 128:(cok + 1) * 128],
                                         ones_b[:], start=False,
                                         stop=(n == total - 1),
                                         skip_group_check=True)
                        n += 1
                return ps

            def conv2d_group(xd, avs, cok):
                """convd (2 f32r) + conv2 36 taps (fp16 weights x bf16/fp16
                spike rings)."""
                ps = pp.tile([128, NN], F32, name="ps2", tag="ps2")
                total = 2 + 36 + (2 if bias2_any else 0)
                n = 0
                for cik in range(2):
                    nc.tensor.matmul(ps[:], wdr[:, (cok * 2 + cik) * 128:]
                                     [:, :128], xd[cik],
                                     start=(n == 0), stop=(n == total - 1),
                                     skip_group_check=True)
                    n += 1
                for cik in range(4):
                    for ky in range(3):
                        for kx in range(3):
                            ti = ky * 3 + kx
                            ws = ((cok * 4 + cik) * 9 + ti) * 128
                            rhs = o1_tap(avs[cik], ky, kx)
                            nc.tensor.matmul(ps[:], w2h[:, ws:ws + 128], rhs,
                                             start=False,
                                             stop=(n == total - 1),
                                             skip_group_check=True)
                            n += 1
                if bias2_any:
                    for b in (b2h, b2l):
                        nc.tensor.matmul(ps[:], b[0:1, cok * 128:(cok + 1) * 128],
                                         ones_r[:], start=False,
                                         stop=(n == total - 1),
                                         skip_group_check=True)
                        n += 1
                return ps

            def o1_views(par):
                return [o1p[k][par] for k in range(4)]

            def o1_tap(tile_, ky, kx):
                base = kx * 420 + ky * 14
                return tile_[:, base:base + 420].rearrange(
                    "p (b f) -> p b f", b=NIMG)[:, :, :196]

            def o1_dense(tile_, b):
                # contiguous 196-elem interior of the kx=1 (unshifted) copy
                return tile_[:, 420 + b * 210 + 14:][:, :196]

            def o1_make_shifts(tile_, b):
                """Replicate the written kx=1 copy into the kx=0/2 copies."""
                src = o1_dense(tile_, b).rearrange("p (y x) -> p y x", y=14)
                d0 = tile_[:, b * 210 + 14:][:, :196].rearrange(
                    "p (y x) -> p y x", y=14)[:, :, 1:14]
                nc.vector.tensor_copy(out=d0, in_=src[:, :, 0:13])
                d2 = tile_[:, 840 + b * 210 + 14:][:, :196].rearrange(
                    "p (y x) -> p y x", y=14)[:, :, 0:13]
                nc.vector.tensor_copy(out=d2, in_=src[:, :, 1:14])

            def scan1(ps_list, t):
                par = t % 2
                for k in range(4):
                    ps = ps_list[k]
                    tl = o1p[k][par]
                    if t == 0:
                        nc.vector.tensor_copy(out=mem1[k][:], in_=ps[:])
                    else:
                        nc.vector.tensor_add(out=mem1[k][:], in0=mem1[k][:],
                                             in1=ps[:])
                    if vth1_c is not None:
                        for b in range(NIMG):
                            m1s = mem1[k][:, b * PIX:(b + 1) * PIX]
                            nc.vector.tensor_scalar(out=o1_dense(tl, b),
                                                    in0=m1s, scalar1=vth1_c,
                                                    scalar2=None, op0=Alu.is_ge)
                        if t == 0:
                            nc.vector.tensor_scalar(out=mask1[k][:],
                                                    in0=mem1[k][:],
                                                    scalar1=vth1_c, scalar2=None,
                                                    op0=Alu.is_ge)
                        else:
                            nc.vector.scalar_tensor_tensor(
                                out=mask1[k][:], in0=mem1[k][:], scalar=vth1_c,
                                in1=mask1[k][:], op0=Alu.is_ge, op1=Alu.max)
                        if t < 3:
                            for b in range(NIMG):
                                m1s = mem1[k][:, b * PIX:(b + 1) * PIX]
                                nc.vector.scalar_tensor_tensor(
                                    out=m1s, in0=o1_dense(tl, b),
                                    scalar=-vth1_c, in1=m1s,
                                    op0=Alu.mult, op1=Alu.add)
                    else:
                        for b in range(NIMG):
                            m1s = mem1[k][:, b * PIX:(b + 1) * PIX]
                            vsv = v1t[:, k * NN + b * PIX:][:, :PIX]
                            nc.vector.tensor_tensor(out=o1_dense(tl, b),
                                                    in0=m1s, in1=vsv,
                                                    op=Alu.is_ge)
                        vs = v1t[:, k * NN:(k + 1) * NN]
                        if t == 0:
                            nc.vector.tensor_tensor(out=mask1[k][:],
                                                    in0=mem1[k][:], in1=vs,
                                                    op=Alu.is_ge)
                        else:
                            sc = iup.tile([128, NN], F32,
                                          name=f"s1_{t}_{k}", tag="iut")
                            nc.vector.tensor_tensor(out=sc[:],
                                                    in0=mem1[k][:], in1=vs,
                                                    op=Alu.is_ge)
                            nc.vector.tensor_max(out=mask1[k][:],
                                                 in0=mask1[k][:], in1=sc[:])
                        if t < 3:
                            for b in range(NIMG):
                                m1s = mem1[k][:, b * PIX:(b + 1) * PIX]
                                vsv = v1t[:, k * NN + b * PIX:][:, :PIX]
                                sc = iup.tile([128, NN], F32,
                                              name=f"s1b_{t}_{k}_{b}", tag="iut")
                                nc.vector.tensor_tensor(
                                    out=sc[:, :PIX], in0=o1_dense(tl, b),
                                    in1=vsv, op=Alu.mult)
                                nc.vector.tensor_sub(out=m1s, in0=m1s,
                                                     in1=sc[:, :PIX])
                    for b in range(NIMG):
                        o1_make_shifts(tl, b)

            def scanF(ps_list, t, pair):
                for k in range(4):
                    ps = ps_list[k]
                    if t == 0:
                        nc.vector.tensor_copy(out=memf[k][:], in_=ps[:])
                    else:
                        nc.vector.tensor_add(out=memf[k][:], in0=memf[k][:],
                                             in1=ps[:])
                    if t < 3:
                        o3_dst = iup.tile([128, NN], F32,
                                          name=f"o3s_{pair}_{t}_{k}",
                                          tag="iut")[:]
                    else:
                        o3t = ostr.tile([128, NN], F32, name=f"o3_{pair}_{k}",
                                        tag="ost")
                        o3_dst = o3t[:]
                    if vthf_c is not None:
                        nc.vector.tensor_scalar(out=o3_dst, in0=memf[k][:],
                                                scalar1=vthf_c, scalar2=None,
                                                op0=Alu.is_ge)
                        if t == 0:
                            nc.vector.tensor_scalar(out=mask3[k][:],
                                                    in0=memf[k][:],
                                                    scalar1=vthf_c, scalar2=None,
                                                    op0=Alu.is_ge)
                        else:
                            nc.vector.scalar_tensor_tensor(
                                out=mask3[k][:], in0=memf[k][:], scalar=vthf_c,
                                in1=mask3[k][:], op0=Alu.is_ge, op1=Alu.max)
                        if t < 3:
                            nc.vector.scalar_tensor_tensor(
                                out=memf[k][:], in0=o3_dst, scalar=-vthf_c,
                                in1=memf[k][:], op0=Alu.mult, op1=Alu.add)
                    else:
                        vs = vft[:, k * NN:(k + 1) * NN]
                        nc.vector.tensor_tensor(out=o3_dst, in0=memf[k][:],
                                                in1=vs, op=Alu.is_ge)
                        if t == 0:
                            nc.vector.tensor_copy(out=mask3[k][:], in_=o3_dst)
                        else:
                            nc.vector.tensor_max(out=mask3[k][:],
                                                 in0=mask3[k][:], in1=o3_dst)
                        if t < 3:
                            nc.vector.tensor_tensor(out=o3_dst, in0=o3_dst,
                                                    in1=vs, op=Alu.mult)
                            nc.vector.tensor_sub(out=memf[k][:], in0=memf[k][:],
                                                 in1=o3_dst)
                    if t == 3:
                        nc.sync.dma_start(
                            out=O3d[pair][:, k * NN:(k + 1) * NN], in_=o3_dst)
                        iut = iup.tile([128, NN], F32,
                                       name=f"iu_{pair}_{k}", tag="iut")
                        nc.scalar.copy(out=iut[:], in_=ps[:])
                        nc.sync.dma_start(out=IUd[pair, k], in_=iut[:])

            # Startup DMA order: pair-0 x(t=0) first, then w1 cok-chunks
            # in fine sub-chunks (a single DMA stream runs ~57 GB/s, so the
            # largest single transfer gates the first matmul), x(t=1),
            # o1p/aap zero rings, wdr, then w2 cok-chunks (first needed
            # ~+100us).
            # p-state warmup: junk matmuls on zeroed tiles keep the PE's HAM
            # clock gate open through the startup DMA window; sized to end
            # at data-arrival: 48 x 392-col then 24 x 64-col fine tail.
            wtmp = st.tile([128, 128], BF16, name="wtmp")
            xtmp = st.tile([128, NN], BF16, name="xtmp")
            nc.vector.memset(wtmp[:], 0.0)
            nc.vector.memset(xtmp[:], 0.0)
            pwarm = pp.tile([128, NN], F32, name="warm", tag="ps1")
            for i in range(22):
                nc.tensor.matmul(pwarm[:], wtmp[:], xtmp[:], start=(i == 0),
                                 stop=False, skip_group_check=True)
            for i in range(16):
                nc.tensor.matmul(pwarm[:, :64], wtmp[:], xtmp[:, :64],
                                 start=False, stop=(i == 15),
                                 skip_group_check=True)
            wrd = iup.tile([128, NN], F32, name="warmrd", tag="iut")
            nc.scalar.copy(out=wrd[:], in_=pwarm[:])

            # First load in bandwidth-priority order: the startup window is
            # HBM-bound (all 16 DMA engines saturate), so stream exactly the
            # first conv group's operands first: xh + w1h cok0, then the
            # next-needed sets in consumption order.
            CW1 = 2 * 9 * 128
            xh0_t, xl0_t, xd0_t = [], [], []
            for cik in range(2):
                xh0_t.append(xp.tile([128, XF], BF16,
                                     name=f"xh_0_0_{cik}", tag=f"xh{cik}"))
                xl0_t.append(xp.tile([128, XF], BF16,
                                     name=f"xl_0_0_{cik}", tag=f"xl{cik}"))
                xd0_t.append(xdp.tile([128, NN], F32R,
                                      name=f"xd_0_0_{cik}", tag=f"xd{cik}"))
            H5 = 5 * NN
            nc.sync.dma_start(out=xh0_t[0][:, :H5], in_=XTHd[0, 0, 0][:, :H5])
            nc.sync.dma_start(out=w1h[:, :CW1], in_=W1Hd[:, :CW1])
            nc.sync.dma_start(out=xh0_t[0][:, H5:], in_=XTHd[0, 0, 0][:, H5:])
            nc.sync.dma_start(out=xh0_t[1][:, :H5], in_=XTHd[0, 0, 1][:, :H5])
            nc.sync.dma_start(out=xh0_t[1][:, H5:], in_=XTHd[0, 0, 1][:, H5:])
            nc.sync.dma_start(out=w1l[:, :CW1], in_=W1Ld[:, :CW1])
            nc.sync.dma_start(out=xl0_t[0][:], in_=XTLd[0, 0, 0])
            nc.sync.dma_start(out=xl0_t[1][:], in_=XTLd[0, 0, 1])
            sl1 = slice(CW1, 2 * CW1)
            nc.sync.dma_start(out=w1h[:, sl1], in_=W1Hd[:, sl1])
            nc.sync.dma_start(out=w1l[:, sl1], in_=W1Ld[:, sl1])
            for cik in range(2):
                nc.sync.dma_start(out=xd0_t[cik][:], in_=XDd[0, 0, cik])
            for k in range(4):
                for par in range(2):
                    nc.vector.memset(o1p[k][par][:], 0.0)
                nc.vector.memset(aap[k][:], 0.0)
            for q in range(2, 4):
                sl = slice(q * CW1, (q + 1) * CW1)
                nc.sync.dma_start(out=w1h[:, sl], in_=W1Hd[:, sl])
                nc.sync.dma_start(out=w1l[:, sl], in_=W1Ld[:, sl])
            xv00 = (xh0_t, xl0_t, xd0_t)
            xv01 = load_x(0, 1)
            nc.sync.dma_start(out=wdr[:], in_=WDRd[:])
            CW2 = 4 * 9 * 128
            for q in range(4):
                sl = slice(q * CW2, (q + 1) * CW2)
                nc.sync.dma_start(out=w2h[:, sl], in_=W2Hd[:, sl])

            for pair in range(NPAIR):
                xv = {}
                if pair == 0:
                    xv[0], xv[1] = xv00, xv01
                else:
                    xv[0] = load_x(pair, 0)
                    xv[1] = load_x(pair, 1)

                ps1 = {0: [conv1_group(xv[0][0], xv[0][1], k) for k in range(4)]}
                scan1(ps1[0], 0)

                ps1[1] = [conv1_group(xv[1][0], xv[1][1], k) for k in range(4)]
                ps2 = {0: [conv2d_group(xv[0][2], o1_views(0), k)
                           for k in range(4)]}
                xv[2] = load_x(pair, 2)
                scan1(ps1[1], 1)
                scanF(ps2[0], 0, pair)

                ps1[2] = [conv1_group(xv[2][0], xv[2][1], k) for k in range(4)]
                ps2[1] = [conv2d_group(xv[1][2], o1_views(1), k)
                          for k in range(4)]
                xv[3] = load_x(pair, 3)
                scan1(ps1[2], 2)
                scanF(ps2[1], 1, pair)

                ps1[3] = [conv1_group(xv[3][0], xv[3][1], k) for k in range(4)]
                ps2[2] = [conv2d_group(xv[2][2], o1_views(0), k)
                          for k in range(4)]
                xc, xcd = load_xc(pair)
                scan1(ps1[3], 3)
                scanF(ps2[2], 2, pair)

                # ANN branch: a = relu(conv1(inp_c)) * mask1 into the fp16
                # rings, then out_c = relu(conv2(a) + convd(inp_c)) * mask3.
                # For the last pair the ANN groups run BEFORE the spike t=3
                # conv2 group so the kernel's final matmul is followed only
                # by the short scanF(3) + mask-mult + DMA tail (the oc relu
                # overlaps the t=3 matmuls; only the mask3 gate must wait).
                def ann_front():
                    ps_c1 = [conv1_group_ann(xc, k) for k in range(4)]
                    for k in range(4):
                        tl = aap[k]
                        sc = iup.tile([128, NN], F32, name=f"ar_{pair}_{k}",
                                      tag="iut")
                        nc.scalar.activation(sc[:], ps_c1[k][:], Act.Relu)
                        for b in range(NIMG):
                            nc.vector.tensor_tensor(
                                out=o1_dense(tl, b),
                                in0=sc[:, b * PIX:(b + 1) * PIX],
                                in1=mask1[k][:, b * PIX:(b + 1) * PIX],
                                op=Alu.mult)
                            o1_make_shifts(tl, b)
                    return [conv2d_group(xcd, aap, k) for k in range(4)]

                def ann_relu(ps_c2):
                    octs = []
                    for k in range(4):
                        oct_ = ostr.tile([128, NN], F32, name=f"oc_{pair}_{k}",
                                         tag=f"oct{k}")
                        nc.scalar.activation(oct_[:], ps_c2[k][:], Act.Relu)
                        octs.append(oct_)
                    return octs

                def ann_store(octs):
                    for k in range(4):
                        oct_ = octs[k]
                        nc.vector.tensor_tensor(out=oct_[:], in0=oct_[:],
                                                in1=mask3[k][:], op=Alu.mult)
                        # b-major 392 cols map contiguously into OCd[k]
                        nc.scalar.dma_start(
                            out=OCd[pair][:, k * NN:(k + 1) * NN],
                            in_=oct_[:])

                if pair == NPAIR - 1:
                    ps_c2 = ann_front()
                    octs = ann_relu(ps_c2)
                    ps2[3] = [conv2d_group(xv[3][2], o1_views(1), k)
                              for k in range(4)]
                    scanF(ps2[3], 3, pair)
                    ann_store(octs)
                else:
                    ps2[3] = [conv2d_group(xv[3][2], o1_views(1), k)
                              for k in range(4)]
                    scanF(ps2[3], 3, pair)
                    ps_c2 = ann_front()
                    ann_store(ann_relu(ps_c2))

    nc.finalize()
    return nc


def _pack_weights(w):
    # cok-major: [128part=cin_low, (cok, cik, tap, 128 cout_low)] so per-cok
    # DMA chunks unlock conv groups incrementally at startup.
    Co, Ci, kh, kw = w.shape
    nchunk = Ci // 128
    a = w.reshape(4, 128, nchunk, 128, kh * kw)  # [cok, co_low, cik, ci_low, tap]
    return np.ascontiguousarray(
        a.transpose(3, 0, 2, 4, 1).reshape(128, 4 * nchunk * kh * kw * 128))


def _vth_const(v):
    v = np.asarray(v, np.float32)
    return float(v.flat[0]) if np.all(v == v.flat[0]) else None


def _vth_rep(v):
    # [512,14,14] -> [128, (chunk, img, pix)] replicated over the image pair
    a = np.asarray(v, np.float32).reshape(4, 128, PIX)
    a = np.broadcast_to(a[:, None, :, :], (4, NIMG, 128, PIX))
    return np.ascontiguousarray(a.transpose(2, 0, 1, 3).reshape(128, 4 * NN))


def _bf_split(a):
    h = a.astype(ml_dtypes.bfloat16)
    l = (a - h.astype(np.float32)).astype(ml_dtypes.bfloat16)
    return np.ascontiguousarray(h), np.ascontiguousarray(l)


def _r_split(a):
    """f32r (11-mantissa-bit) hi/lo split, both stored as fp32 bits."""
    i = a.astype(np.float32).view(np.uint32).astype(np.uint64)
    i = (i + ((i >> 12) & 1) + 0x7FF) & np.uint64(0xFFFFF000)
    h = i.astype(np.uint32).view(np.float32)
    return np.ascontiguousarray(h), np.ascontiguousarray(a - h)


def _taps(x):
    """[..., 28, 28] -> [..., 9, 196]: per-tap stride-2 window extraction
    from the zero-padded 30x30 frame (fully contiguous per-tap layout)."""
    sh = x.shape[:-2]
    xp = np.zeros(sh + (30, 30), x.dtype)
    xp[..., 1:29, 1:29] = x
    out = np.empty(sh + (9, 196), x.dtype)
    for ky in range(3):
        for kx in range(3):
            out[..., ky * 3 + kx, :] = \
                xp[..., ky:ky + 28:2, kx:kx + 28:2].reshape(sh + (196,))
    return out


def kernel(inp_s, inp_u, inp_c, conv1_w, conv2_w, ds_w,
           bn1_gamma, bn1_beta, bn1_mean, bn1_var,
           bn2_gamma, bn2_beta, bn2_mean, bn2_var,
           dsbn_gamma, dsbn_beta, dsbn_mean, dsbn_var,
           vth1, vth2, vth_ds, vth_if):
    global LAST_RESULT
    f32 = lambda x: np.asarray(x, np.float32)
    inp_s, inp_c = f32(inp_s), f32(inp_c)

    def fold(w, gamma, beta, mean, var):
        s = f32(gamma) / np.sqrt(f32(var) + np.float32(EPS))
        return f32(w) * s[:, None, None, None], f32(beta) - f32(mean) * s

    w1, b1 = fold(conv1_w, bn1_gamma, bn1_beta, bn1_mean, bn1_var)
    w2, b2 = fold(conv2_w, bn2_gamma, bn2_beta, bn2_mean, bn2_var)
    wd, bd = fold(ds_w, dsbn_gamma, dsbn_beta, dsbn_mean, dsbn_var)
    b2d = b2 + bd

    vth1_c = _vth_const(vth1)
    vthf_c = _vth_const(vth_if)
    bias1_any = bool(np.any(b1 != 0))
    bias2_any = bool(np.any(b2d != 0))

    cfg = (bias1_any, bias2_any, vth1_c, vthf_c)
    if cfg not in _CACHE:
        _CACHE[cfg] = _build(cfg)
    nc = _CACHE[cfg]

    W1H, W1L = _bf_split(_pack_weights(w1))
    W2H = _pack_weights(w2).astype(np.float16)
    WDR = _pack_weights(wd)

    T, B = inp_s.shape[:2]
    # bf16 hi/lo split, then per-tap extraction (zeros pad -> zeros)
    hi_s = inp_s.astype(ml_dtypes.bfloat16)
    lo_s = (inp_s - hi_s.astype(np.float32)).astype(ml_dtypes.bfloat16)
    th_s = _taps(hi_s)                       # [T,B,256,9,196] bf16
    tl_s = _taps(lo_s)
    xd_s = np.ascontiguousarray(inp_s[..., 0::2, 0::2].reshape(T, B, 256, 196))
    tc_c = _taps(inp_c.astype(ml_dtypes.bfloat16))   # [B,256,9,196]
    xcd_c = np.ascontiguousarray(inp_c[..., 0::2, 0::2].reshape(B, 256, 196))

    def pack_xt(a):
        # [T, 4img, 256, 9, 196] -> [NPAIR, T, 2cik, 128, tap, img, 196]
        r = a.reshape(T, NPAIR, NIMG, 2, 128, 9, 196)
        return np.ascontiguousarray(
            r.transpose(1, 0, 3, 4, 5, 2, 6).reshape(NPAIR, T, 2, 128, XF))

    def pack_xct(a):
        r = a.reshape(NPAIR, NIMG, 2, 128, 9, 196)
        return np.ascontiguousarray(
            r.transpose(0, 2, 3, 4, 1, 5).reshape(NPAIR, 2, 128, XF))

    def pack_xd(a, f):
        # [T, 4img, 256, f] -> [NPAIR, T, 2cik, 128, NIMG*f]
        r = a.reshape(T, NPAIR, NIMG, 2, 128, f)
        return np.ascontiguousarray(
            r.transpose(1, 0, 3, 4, 2, 5).reshape(NPAIR, T, 2, 128, NIMG * f))

    def pack_xcd(a, f):
        r = a.reshape(NPAIR, NIMG, 2, 128, f)
        return np.ascontiguousarray(
            r.transpose(0, 2, 3, 1, 4).reshape(NPAIR, 2, 128, NIMG * f))

    in_maps = []
    for core in range(NCORES):
        b0 = core * BPC
        m = {
            "W1H": W1H, "W1L": W1L, "W2H": W2H, "WDR": WDR,
            "XTH": pack_xt(th_s[:, b0:b0 + BPC]),
            "XTL": pack_xt(tl_s[:, b0:b0 + BPC]),
            "XD": pack_xd(xd_s[:, b0:b0 + BPC], 196),
            "XC": pack_xct(tc_c[b0:b0 + BPC]),
            "XCD": pack_xcd(xcd_c[b0:b0 + BPC], 196),
        }
        if bias1_any:
            bh, bl = _bf_split(b1.reshape(1, 512))
            m["B1H"], m["B1L"] = bh, bl
        if bias2_any:
            bh, bl = _r_split(b2d.reshape(1, 512))
            m["B2H"], m["B2L"] = bh, bl
            m["ONER"] = np.ones((1, NN), np.float32)
        if vth1_c is None:
            m["VTH1R"] = _vth_rep(vth1)
        if vthf_c is None:
            m["VTHFR"] = _vth_rep(vth_if)
        in_maps.append(m)

    from concourse.bass_utils import run_bass_kernel_spmd
    # Install the NTFF profile hook shim unconditionally: bass_utils imports
    # antenv.axon_hooks unguarded whenever tracing is requested (including
    # via the BASS_TRACE env var, outside our control), and this container's
    # antenv lacks that submodule.
    if True:
        try:
            import sys
            import types
            try:
                from antenv.axon_hooks import set_axon_ntff_profile_hook
            except ImportError:
                # container's antenv lacks axon_hooks; synthesize it so
                # bass_utils' unguarded import under trace=True works
                import antenv
                mod = types.ModuleType("antenv.axon_hooks")
                mod._hook = None

                def set_axon_ntff_profile_hook(h, _m=mod):
                    _m._hook = h

                def get_axon_ntff_profile_hook(_m=mod):
                    return _m._hook

                mod.set_axon_ntff_profile_hook = set_axon_ntff_profile_hook
                mod.get_axon_ntff_profile_hook = get_axon_ntff_profile_hook
                sys.modules["antenv.axon_hooks"] = mod
                antenv.axon_hooks = mod
            from trn_agent_boot.trn_boot import _ntff_profile_via_ctypes
            set_axon_ntff_profile_hook(
                _ntff_profile_via_ctypes('/opt/axon/libaxon_pjrt.so'))
        except Exception:
            pass
    res = run_bass_kernel_spmd(nc, in_maps, core_ids=list(range(NCORES)),
                               trace=TRACE)
    LAST_RESULT = res

    o3 = np.empty((B, 512, 14, 14), np.float32)
    iu = np.empty((B, 512, 14, 14), np.float32)
    oc = np.empty((B, 512, 14, 14), np.float32)
    for core in range(NCORES):
        b0 = core * BPC
        for name, dst in (("O3", o3), ("OC", oc)):
            arr = res.results[core][name].reshape(NPAIR, 128, 4, NIMG, PIX)
            arr = arr.transpose(0, 3, 2, 1, 4).reshape(BPC, 512, 14, 14)
            dst[b0:b0 + BPC] = arr
        arr = res.results[core]["IU"].reshape(NPAIR, 4, 128, NIMG, PIX)
        arr = arr.transpose(0, 3, 1, 2, 4).reshape(BPC, 512, 14, 14)
        iu[b0:b0 + BPC] = arr
    return o3, iu, oc
